# revision 4
# baseline (speedup 1.0000x reference)
"""Trainium2 Bass kernel for the KGCN-style neighbor aggregator.

Math (per batch b, entity e):
    scores[n] = sum_d u[b,d] * R[b,e,n,d]          # relation attention logits
    p = softmax(scores)                            # over N neighbors
    agg[d]    = sum_n p[n] * V[b,e,n,d]            # weighted neighbor sum
    out       = relu(W @ concat(self, agg) + bias) # Linear(2D -> D)

Sharding: pure data parallel over the batch dim B=1024 across 8 cores
(128 batches/core); the Linear weights are replicated. No collectives.

Layout on each core: batch-on-partitions tiles [128, N*D] per entity.
DVE does the two big elementwise multiplies and the two segmented
reductions; ScalarE does exp (with fused denominator accumulation) and
PSUM->SBUF copies; TensorE does the final Linear via transposes and a
two-matmul PSUM accumulation group.

Softmax max-subtraction is skipped: scores ~ N(0, 64) so |s| > 88 (the
fp32 exp overflow point) is a >11-sigma event.
"""

import sys

import numpy as np

sys.path.insert(0, "/opt/trn_rl_repo")

import concourse.bacc as bacc  # noqa: E402
import concourse.bass as bass  # noqa: E402
import concourse.mybir as mybir  # noqa: E402
import concourse.tile as tile  # noqa: E402
from concourse.bass_utils import run_bass_kernel_spmd  # noqa: E402

B, E, N, D = 1024, 64, 32, 64
NCORES = 8
BC = B // NCORES  # 128 batches per core
F32 = mybir.dt.float32
AX = mybir.AxisListType
OP = mybir.AluOpType
ACT = mybir.ActivationFunctionType

_CACHE = {}


def _build_nc() -> bass.Bass:
    # Bacc (not raw Bass): its finalize() runs generate_event_semaphores(),
    # which splits multi-wait sync conditions onto InstEventSemaphore —
    # hardware allows only one sync wait per instruction.
    nc = bacc.Bacc()

    nr_d = nc.declare_dram_parameter("nr", [BC, E, N, D], F32, isOutput=False)
    nv_d = nc.declare_dram_parameter("nv", [BC, E, N, D], F32, isOutput=False)
    sv_d = nc.declare_dram_parameter("sv", [BC, E, 1, D], F32, isOutput=False)
    u_d = nc.declare_dram_parameter("u", [BC, D], F32, isOutput=False)
    w1t_d = nc.declare_dram_parameter("w1t", [D, D], F32, isOutput=False)
    w2t_d = nc.declare_dram_parameter("w2t", [D, D], F32, isOutput=False)
    bias_d = nc.declare_dram_parameter("bias", [D, 1], F32, isOutput=False)
    ident_d = nc.declare_dram_parameter("ident", [128, 128], F32, isOutput=False)
    y_d = nc.declare_dram_parameter("y", [BC, E, D], F32, isOutput=True)

    with tile.TileContext(nc) as tc:
        with (
            tc.tile_pool(name="consts", bufs=1) as consts,
            tc.tile_pool(name="loads", bufs=3) as loads,
            tc.tile_pool(name="work", bufs=2) as work,
            tc.tile_pool(name="small", bufs=3) as small,
            tc.tile_pool(name="psum", bufs=2, space="PSUM") as psum,
        ):
            # One-time constant loads.
            u_sb = consts.tile([BC, D], F32)
            nc.sync.dma_start(u_sb[:], u_d[:])
            w1t_sb = consts.tile([D, D], F32)
            nc.sync.dma_start(w1t_sb[:], w1t_d[:])
            w2t_sb = consts.tile([D, D], F32)
            nc.sync.dma_start(w2t_sb[:], w2t_d[:])
            bias_sb = consts.tile([D, 1], F32)
            nc.sync.dma_start(bias_sb[:], bias_d[:])
            ident_sb = consts.tile([128, 128], F32)
            nc.sync.dma_start(ident_sb[:], ident_d[:])

            u_bc = u_sb[:].unsqueeze(1).broadcast_to((BC, N, D))

            for e in range(E):
                # Loads for this entity: [128 b, N*D] with 8KB contiguous runs.
                rt = loads.tile([BC, N * D], F32, tag="rt")
                nc.sync.dma_start(rt[:], nr_d[:, e])
                vt = loads.tile([BC, N * D], F32, tag="vt")
                nc.sync.dma_start(vt[:], nv_d[:, e])
                st = small.tile([BC, D], F32, tag="st")
                nc.sync.dma_start(st[:], sv_d[:, e, 0, :])

                # scores[b, n] = sum_d R[b, n, d] * u[b, d]
                tmp = work.tile([BC, N * D], F32, tag="tmp")
                nc.vector.tensor_mul(tmp[:], rt[:], u_bc)
                scr = small.tile([BC, N], F32, tag="scr")
                nc.vector.reduce_sum(
                    scr[:], tmp[:].rearrange("b (n d) -> b n d", n=N), axis=AX.X
                )

                # softmax over n (no max subtraction; see module docstring)
                w_t = small.tile([BC, N], F32, tag="w_t")
                den = small.tile([BC, 1], F32, tag="den")
                nc.scalar.activation(w_t[:], scr[:], ACT.Exp, accum_out=den[:])
                rden = small.tile([BC, 1], F32, tag="rden")
                nc.vector.reciprocal(rden[:], den[:])
                p_t = small.tile([BC, N], F32, tag="p_t")
                nc.vector.tensor_scalar_mul(p_t[:], w_t[:], rden[:])

                # agg[b, d] = sum_n p[b, n] * V[b, n, d]
                tmp2 = work.tile([BC, N * D], F32, tag="tmp2")
                nc.vector.tensor_mul(
                    tmp2[:], vt[:], p_t[:].unsqueeze(2).broadcast_to((BC, N, D))
                )
                agg = small.tile([BC, D], F32, tag="agg")
                nc.vector.reduce_sum(
                    agg[:], tmp2[:].rearrange("b (n d) -> b d n", n=N), axis=AX.X
                )

                # Linear(2D -> D): out2[dout, b] = W1t.T @ self.T + W2t.T @ agg.T
                selfT_ps = psum.tile([D, BC], F32, tag="selfT_ps")
                nc.tensor.transpose(selfT_ps[:], st[:], ident_sb[:])
                selfT = small.tile([D, BC], F32, tag="selfT")
                nc.scalar.copy(selfT[:], selfT_ps[:])

                aggT_ps = psum.tile([D, BC], F32, tag="aggT_ps")
                nc.tensor.transpose(aggT_ps[:], agg[:], ident_sb[:])
                aggT = small.tile([D, BC], F32, tag="aggT")
                nc.scalar.copy(aggT[:], aggT_ps[:])

                out2_ps = psum.tile([D, BC], F32, tag="out2_ps")
                nc.tensor.matmul(out2_ps[:], w1t_sb[:], selfT[:], start=True, stop=False)
                nc.tensor.matmul(out2_ps[:], w2t_sb[:], aggT[:], start=False, stop=True)
                out2 = small.tile([D, BC], F32, tag="out2")
                nc.scalar.activation(
                    out2[:], out2_ps[:], ACT.Relu, bias=bias_sb[:, 0:1]
                )

                # Transpose back to [b, dout] and store.
                yT_ps = psum.tile([BC, D], F32, tag="yT_ps")
                nc.tensor.transpose(yT_ps[:], out2[:], ident_sb[:D, :D])
                y_sb = small.tile([BC, D], F32, tag="y_sb")
                nc.scalar.copy(y_sb[:], yT_ps[:])
                nc.sync.dma_start(y_d[:, e, :], y_sb[:])

    nc.finalize()
    return nc


def _get_nc() -> bass.Bass:
    if "nc" not in _CACHE:
        _CACHE["nc"] = _build_nc()
    return _CACHE["nc"]


def _prep_inputs(self_vectors, neighbor_vectors, neighbor_relations, user_embeddings, W, b):
    f32 = np.float32
    sv = np.ascontiguousarray(np.asarray(self_vectors, dtype=f32))
    nv = np.ascontiguousarray(np.asarray(neighbor_vectors, dtype=f32))
    nr = np.ascontiguousarray(np.asarray(neighbor_relations, dtype=f32))
    u = np.ascontiguousarray(np.asarray(user_embeddings, dtype=f32))
    W = np.asarray(W, dtype=f32)
    b = np.asarray(b, dtype=f32)
    w1t = np.ascontiguousarray(W[:, :D].T)
    w2t = np.ascontiguousarray(W[:, D:].T)
    bias = np.ascontiguousarray(b.reshape(D, 1))
    ident = np.eye(128, dtype=f32)

    in_maps = []
    for c in range(NCORES):
        s = slice(c * BC, (c + 1) * BC)
        in_maps.append(
            {
                "nr": nr[s],
                "nv": nv[s],
                "sv": sv[s],
                "u": u[s],
                "w1t": w1t,
                "w2t": w2t,
                "bias": bias,
                "ident": ident,
            }
        )
    return in_maps


def run(inputs: dict, trace: bool = False):
    """Run the SPMD kernel; returns the BassKernelResults."""
    in_maps = _prep_inputs(**inputs)
    return run_bass_kernel_spmd(
        _get_nc(), in_maps, core_ids=list(range(NCORES)), trace=trace
    )


def kernel(**inputs) -> np.ndarray:
    res = run(inputs, trace=False)
    return np.concatenate([r["y"] for r in res.results], axis=0)


# revision 8
# speedup vs baseline: 1.0030x; 1.0030x over previous
"""Trainium2 Bass kernel for the KGCN-style neighbor aggregator.

Math (per batch b, entity e):
    scores[n] = sum_d u[b,d] * R[b,e,n,d]          # relation attention logits
    p = softmax(scores)                            # over N neighbors
    agg[d]    = sum_n p[n] * V[b,e,n,d]            # weighted neighbor sum
    out       = relu(W @ concat(self, agg) + bias) # Linear(2D -> D)

Sharding: pure data parallel over the batch dim B=1024 across 8 cores
(128 batches/core); the Linear weights are replicated. No collectives.

Layout on each core: batch-on-partitions tiles [128, N*D] per entity.
DVE does the two big elementwise multiplies and the two segmented
reductions; ScalarE does exp (with fused denominator accumulation) and
PSUM->SBUF copies; TensorE does the final Linear via transposes and a
two-matmul PSUM accumulation group.

Softmax max-subtraction is skipped: scores ~ N(0, 64) so |s| > 88 (the
fp32 exp overflow point) is a >11-sigma event.
"""

import sys

import numpy as np

sys.path.insert(0, "/opt/trn_rl_repo")

import concourse.bacc as bacc  # noqa: E402
import concourse.bass as bass  # noqa: E402
import concourse.mybir as mybir  # noqa: E402
import concourse.tile as tile  # noqa: E402
from concourse.bass_utils import run_bass_kernel_spmd  # noqa: E402

B, E, N, D = 1024, 64, 32, 64
NCORES = 8
BC = B // NCORES  # 128 batches per core
F32 = mybir.dt.float32
AX = mybir.AxisListType
OP = mybir.AluOpType
ACT = mybir.ActivationFunctionType

_CACHE = {}


def _build_nc(repeat: int = 1) -> bass.Bass:
    # Bacc (not raw Bass): its finalize() runs generate_event_semaphores(),
    # which splits multi-wait sync conditions onto InstEventSemaphore —
    # hardware allows only one sync wait per instruction.
    nc = bacc.Bacc()

    nr_d = nc.declare_dram_parameter("nr", [BC, E, N, D], F32, isOutput=False)
    nv_d = nc.declare_dram_parameter("nv", [BC, E, N, D], F32, isOutput=False)
    sv_d = nc.declare_dram_parameter("sv", [BC, E, 1, D], F32, isOutput=False)
    u_d = nc.declare_dram_parameter("u", [BC, D], F32, isOutput=False)
    w1t_d = nc.declare_dram_parameter("w1t", [D, D], F32, isOutput=False)
    w2t_d = nc.declare_dram_parameter("w2t", [D, D], F32, isOutput=False)
    bias_d = nc.declare_dram_parameter("bias", [D, 1], F32, isOutput=False)
    ident_d = nc.declare_dram_parameter("ident", [128, 128], F32, isOutput=False)
    y_d = nc.declare_dram_parameter("y", [BC, E, D], F32, isOutput=True)

    with tile.TileContext(nc) as tc:
        with (
            tc.tile_pool(name="consts", bufs=1) as consts,
            tc.tile_pool(name="loads", bufs=3) as loads,
            tc.tile_pool(name="work", bufs=2) as work,
            tc.tile_pool(name="small", bufs=3) as small,
            tc.tile_pool(name="psum", bufs=2, space="PSUM") as psum,
        ):
            # One-time constant loads.
            u_sb = consts.tile([BC, D], F32)
            nc.sync.dma_start(u_sb[:], u_d[:])
            w1t_sb = consts.tile([D, D], F32)
            nc.sync.dma_start(w1t_sb[:], w1t_d[:])
            w2t_sb = consts.tile([D, D], F32)
            nc.sync.dma_start(w2t_sb[:], w2t_d[:])
            bias_sb = consts.tile([D, 1], F32)
            nc.sync.dma_start(bias_sb[:], bias_d[:])
            ident_sb = consts.tile([128, 128], F32)
            nc.sync.dma_start(ident_sb[:], ident_d[:])

            u_bc = u_sb[:].unsqueeze(1).broadcast_to((BC, N, D))

            def body():
                for e in range(E):
                    _entity(nc, tc, loads, work, small, psum, e,
                            nr_d, nv_d, sv_d, y_d, u_bc,
                            w1t_sb, w2t_sb, bias_sb, ident_sb)

            if repeat == 1:
                body()
            else:
                # Benchmark mode: re-run the whole sweep `repeat` times on
                # device so wall-clock deltas isolate on-device exec time.
                with tc.For_i(0, repeat, 1):
                    body()

    nc.finalize()
    return nc


def _entity(nc, tc, loads, work, small, psum, e,
            nr_d, nv_d, sv_d, y_d, u_bc, w1t_sb, w2t_sb, bias_sb, ident_sb):
    if True:
            if True:
                # Loads for this entity: [128 b, N*D] with 8KB contiguous runs.
                rt = loads.tile([BC, N * D], F32, tag="rt")
                nc.sync.dma_start(rt[:], nr_d[:, e])
                vt = loads.tile([BC, N * D], F32, tag="vt")
                nc.sync.dma_start(vt[:], nv_d[:, e])
                st = small.tile([BC, D], F32, tag="st")
                nc.sync.dma_start(st[:], sv_d[:, e, 0, :])

                # scores[b, n] = sum_d R[b, n, d] * u[b, d]
                tmp = work.tile([BC, N * D], F32, tag="tmp")
                nc.vector.tensor_mul(tmp[:], rt[:], u_bc)
                scr = small.tile([BC, N], F32, tag="scr")
                nc.vector.reduce_sum(
                    scr[:], tmp[:].rearrange("b (n d) -> b n d", n=N), axis=AX.X
                )

                # softmax over n (no max subtraction; see module docstring)
                w_t = small.tile([BC, N], F32, tag="w_t")
                den = small.tile([BC, 1], F32, tag="den")
                nc.scalar.activation(w_t[:], scr[:], ACT.Exp, accum_out=den[:])
                rden = small.tile([BC, 1], F32, tag="rden")
                nc.vector.reciprocal(rden[:], den[:])
                p_t = small.tile([BC, N], F32, tag="p_t")
                nc.vector.tensor_scalar_mul(p_t[:], w_t[:], rden[:])

                # agg[b, d] = sum_n p[b, n] * V[b, n, d]
                tmp2 = work.tile([BC, N * D], F32, tag="tmp2")
                nc.vector.tensor_mul(
                    tmp2[:], vt[:], p_t[:].unsqueeze(2).broadcast_to((BC, N, D))
                )
                agg = small.tile([BC, D], F32, tag="agg")
                nc.vector.reduce_sum(
                    agg[:], tmp2[:].rearrange("b (n d) -> b d n", n=N), axis=AX.X
                )

                # Linear(2D -> D): out2[dout, b] = W1t.T @ self.T + W2t.T @ agg.T
                selfT_ps = psum.tile([D, BC], F32, tag="selfT_ps")
                nc.tensor.transpose(selfT_ps[:], st[:], ident_sb[:])
                selfT = small.tile([D, BC], F32, tag="selfT")
                nc.scalar.copy(selfT[:], selfT_ps[:])

                aggT_ps = psum.tile([D, BC], F32, tag="aggT_ps")
                nc.tensor.transpose(aggT_ps[:], agg[:], ident_sb[:])
                aggT = small.tile([D, BC], F32, tag="aggT")
                nc.scalar.copy(aggT[:], aggT_ps[:])

                out2_ps = psum.tile([D, BC], F32, tag="out2_ps")
                nc.tensor.matmul(out2_ps[:], w1t_sb[:], selfT[:], start=True, stop=False)
                nc.tensor.matmul(out2_ps[:], w2t_sb[:], aggT[:], start=False, stop=True)
                out2 = small.tile([D, BC], F32, tag="out2")
                nc.scalar.activation(
                    out2[:], out2_ps[:], ACT.Relu, bias=bias_sb[:, 0:1]
                )

                # Transpose back to [b, dout] and store.
                yT_ps = psum.tile([BC, D], F32, tag="yT_ps")
                nc.tensor.transpose(yT_ps[:], out2[:], ident_sb[:D, :D])
                y_sb = small.tile([BC, D], F32, tag="y_sb")
                nc.scalar.copy(y_sb[:], yT_ps[:])
                nc.sync.dma_start(y_d[:, e, :], y_sb[:])


def _get_nc(repeat: int = 1) -> bass.Bass:
    key = ("nc", repeat)
    if key not in _CACHE:
        _CACHE[key] = _build_nc(repeat)
    return _CACHE[key]


def _prep_inputs(self_vectors, neighbor_vectors, neighbor_relations, user_embeddings, W, b):
    f32 = np.float32
    sv = np.ascontiguousarray(np.asarray(self_vectors, dtype=f32))
    nv = np.ascontiguousarray(np.asarray(neighbor_vectors, dtype=f32))
    nr = np.ascontiguousarray(np.asarray(neighbor_relations, dtype=f32))
    u = np.ascontiguousarray(np.asarray(user_embeddings, dtype=f32))
    W = np.asarray(W, dtype=f32)
    b = np.asarray(b, dtype=f32)
    w1t = np.ascontiguousarray(W[:, :D].T)
    w2t = np.ascontiguousarray(W[:, D:].T)
    bias = np.ascontiguousarray(b.reshape(D, 1))
    ident = np.eye(128, dtype=f32)

    in_maps = []
    for c in range(NCORES):
        s = slice(c * BC, (c + 1) * BC)
        in_maps.append(
            {
                "nr": nr[s],
                "nv": nv[s],
                "sv": sv[s],
                "u": u[s],
                "w1t": w1t,
                "w2t": w2t,
                "bias": bias,
                "ident": ident,
            }
        )
    return in_maps


def run(inputs: dict, trace: bool = False, repeat: int = 1):
    """Run the SPMD kernel; returns the BassKernelResults."""
    in_maps = _prep_inputs(**inputs)
    return run_bass_kernel_spmd(
        _get_nc(repeat), in_maps, core_ids=list(range(NCORES)), trace=trace
    )


def kernel(**inputs) -> np.ndarray:
    res = run(inputs, trace=False)
    return np.concatenate([r["y"] for r in res.results], axis=0)
